# revision 38
# baseline (speedup 1.0000x reference)
"""Trainium2 Bass kernel for nn_AttentionBlock (B=4, S=2048, E=512, H=8).

Sharding (no cross-core communication):
  core c owns batch b = c//2 and output rows for tokens half = c%2
  (1024 tokens).  Each core computes Q for its own 1024 tokens and K/V for
  the full batch (2048 tokens); K/V projections are duplicated between the
  two cores of a batch (cheap) so attention and the output projection are
  fully local.

Host-side layout prep (inside kernel(), numpy only):
  - x[b] is permuted so the core's own 1024 query tokens are rows 0:1024,
    then transposed to feature-major xkv_t [512, 2048].  Softmax/PV are
    permutation-invariant in the key axis, so K/V token order is irrelevant.
  - weights are passed as W.T [E, E] so lhsT/rhs tiles are direct DMA loads.
  - x and weights are cast to bf16; biases, PSUM accumulation, softmax
    statistics and the normalization stay fp32 (V/out bias rows are bf16
    broadcast tiles added on the DVE — error ~2e-4 absolute, negligible).

Device dataflow (per core):
  The exp of the score matrix (16.8M elements/core) saturates ScalarE if it
  owns all of it, so the exp is split: even head's tile on ScalarE (exact
  LUT exp), odd head's on VectorE via a Schraudolph bit-trick (bf16 bits =
  int16(A*s + B), one fused tensor_scalar from PSUM; numerator and
  denominator use the same approximate weights so softmax normalization
  cancels nearly all the error — measured end-to-end rel-err 2.9e-3).

  A) ~3.5us of junk warmup matmuls (HAM clock un-throttle) under the input
     DMAs, then projections: K/Q feature chunk m=0, V (16 token tiles, DVE
     bias add), K/Q m=1..3.  All gelus precede all exps so the ACT table
     set loads exactly 3 times (gelu, exp, gelu).
  B) attention, q in 512-column halves, head pairs hp inside: per (hp,q,jp)
     unit two K=64 S^T matmuls per head interleaved even/odd so adjacent
     matmuls land in disjoint PE row groups (auto-pack, 2x), exp split
     ACT/DVE, PV accumulates O^T_unnorm + sumexp via [V | 1] lhsT (M=65).
     The stream is software-pipelined with a one-unit lag (each unit's PV
     matmuls are emitted after the next unit's S^T) so the PE never waits
     on the exp of the tile it just produced.
  C) normalization incrementally (q0 batched under q1's attention; q1 in
     hp chunks as drains land, with the tail reciprocals emitted inside the
     drains): reciprocal_approx_fast on sumexp rows (base partition 0 only
     — the custom DVE op breaks at other bases), K=1 selector matmuls
     broadcast across 64 partitions (separate accumulation groups per lhsT
     base partition — mixing crashes the HW), DVE multiply.
  D) out = gelu(ORT chunks @ Wo.T + bo) from PSUM (out tiles 0-3 emitted
     ahead of the tail norm chain); stores alternate sync/scalar queues.
"""

import numpy as np

import concourse.bass as bass
import concourse.tile as tile
from concourse import bacc, mybir

F32 = mybir.dt.float32
BF16 = mybir.dt.bfloat16
I16 = mybir.dt.int16
AF = mybir.ActivationFunctionType
ALU = mybir.AluOpType

E = 512          # embed dim
H = 8            # heads
D = 64           # head dim
P = 128          # partitions
EC = E // P      # 128-row chunks of the embed dim
B = 4
S = 2048
N_CORES = 8
SCALE = 0.125    # 1/sqrt(D)

# Schraudolph exp-as-bit-trick constants: bf16 bits of exp(s/8) ~=
# int16(SCH_A*s + SCH_B).  C=0.045 centers the mantissa sawtooth error.
SCH_A = 128.0 * SCALE * np.log2(np.e)       # 23.08312...
SCH_B = 128.0 * (127.0 - 0.045)             # 16250.24


def build(tc, io, T_KV, T_Q, mm_dt=BF16):
    """Emit the per-core program.  T_KV: key/value tokens; T_Q: query tokens
    (the first T_Q columns of xkv_t)."""
    nc = tc.nc
    n_g = T_KV // 512        # 512-token groups for projections
    n_qg = T_Q // 512        # q groups (S^T rhs is N=512)
    n_kt = T_KV // P         # 128-token key tiles
    assert T_KV % 1024 == 0 and T_Q % 1024 == 0

    xkv = io["xkv_t"]
    out = io["out"]

    with tc.tile_pool(name="persist", bufs=1) as persist, \
         tc.tile_pool(name="ps", space="PSUM", bufs=1) as psp, \
         tc.tile_pool(name="projw", bufs=1) as projw, \
         tc.tile_pool(name="xtp", bufs=2) as xtp, \
         tc.tile_pool(name="ep", bufs=6) as ep, \
         tc.tile_pool(name="tmpp", bufs=3) as tmpp, \
         tc.tile_pool(name="outp", bufs=3) as outp:
        # ---------------- persistent SBUF state ----------------
        wo_sb = [persist.tile([P, E], mm_dt, name=f"wo_sb{k}", tag=f"wo{k}")
                 for k in range(EC)]
        QT = [persist.tile([P, T_Q], mm_dt, name=f"qt_sb{m}", tag=f"qt{m}")
              for m in range(EC)]
        KT = [persist.tile([P, T_KV], mm_dt, name=f"kt_sb{m}", tag=f"kt{m}")
              for m in range(EC)]
        V3 = [persist.tile([P, H, 66], mm_dt, name=f"v3_sb{t}", tag=f"v3{t}")
              for t in range(n_kt)]
        ORT = [persist.tile([P, T_Q], mm_dt, name=f"ort_sb{m}", tag=f"ort{m}")
               for m in range(EC)]
        # sumexp rows live at partition 64 (even head) and 32 (odd head):
        # SBUF accesses may only start at partitions {0, 32, 64, 96} and
        # matmul operand base partitions only at {0, 32, 64}.
        SE = [persist.tile([65, T_Q], F32, name=f"se_sb{hp}", tag=f"se{hp}")
              for hp in range(H // 2)]
        # sumexp rows gathered per head pair so every DVE reciprocal runs at
        # base partition 0 (compute-engine SBUF accesses must start at
        # partition {0,32,64,96}, and the custom reciprocal op is only
        # proven at base 0).
        G = [persist.tile([2, T_Q], F32, name=f"g_se{i}") for i in range(4)]
        g_loc = lambda h: (G[h // 2], h % 2)
        sel2 = persist.tile([65, P], F32, name="sel2")
        bv_bc = persist.tile([P, E], mm_dt, name="bv_bc")
        bo_bc = persist.tile([P, E], mm_dt, name="bo_bc")

        # sel2 rows 64 / 32 are 64-wide ones rows used as K=1 lhsT to
        # broadcast the even / odd head's reciprocal sumexp across 64
        # output partitions.
        nc.vector.memset(sel2[64:65, 0:64], 1.0)
        nc.vector.memset(sel2[32:33, 0:64], 1.0)
        # bf16 strided writes (ACT out / gpsimd memset with a gappy AP)
        # wedge the device — memset the whole tile (contiguous) instead;
        # the V gelu lands via a contiguous staging tile + strided DMA.
        for t in range(n_kt):
            nc.gpsimd.memset(V3[t], 1.0)

        # ---------------- weight / x DMA issue ----------------
        wq_sb = [projw.tile([P, E], mm_dt, name=f"wq_sb{k}", tag=f"wq{k}")
                 for k in range(EC)]
        wk_sb = [projw.tile([P, E], mm_dt, name=f"wk_sb{k}", tag=f"wk{k}")
                 for k in range(EC)]
        wv_sb = [projw.tile([P, E], mm_dt, name=f"wv_sb{k}", tag=f"wv{k}")
                 for k in range(EC)]
        bq_sb = [projw.tile([P, 1], F32, name=f"bq_sb{m}", tag=f"bq{m}")
                 for m in range(EC)]
        bk_sb = [projw.tile([P, 1], F32, name=f"bk_sb{m}", tag=f"bk{m}")
                 for m in range(EC)]
        # x tiles merged to [P, 1024] (one DMA per (group pair, k-chunk));
        # issued on the scalar / vector queues so the startup loads don't
        # serialize behind the weight loads on sync.
        xts = {}
        for gp in range(n_g // 2):
            for k in range(EC):
                xts[gp, k] = xtp.tile([P, 1024], mm_dt, name=f"xt_g{gp}k{k}",
                                      tag=f"xt_{k}")
        for k in range(EC):
            nc.scalar.dma_start(out=xts[0, k],
                                in_=xkv[k * P:(k + 1) * P, 0:1024])
        for k in range(EC):
            nc.scalar.dma_start(out=xts[1, k],
                                in_=xkv[k * P:(k + 1) * P, 1024:2048])
        for k in range(EC):
            nc.sync.dma_start(out=wk_sb[k],
                              in_=io["wk_t"][k * P:(k + 1) * P, :])
        for k in range(EC):
            nc.sync.dma_start(out=wq_sb[k],
                              in_=io["wq_t"][k * P:(k + 1) * P, :])
        for k in range(EC):
            nc.sync.dma_start(out=bq_sb[k],
                              in_=io["bq_col"][k * P:(k + 1) * P, :])
            nc.sync.dma_start(out=bk_sb[k],
                              in_=io["bk_col"][k * P:(k + 1) * P, :])
        for k in range(EC):
            nc.sync.dma_start(out=wv_sb[k],
                              in_=io["wv_t"][k * P:(k + 1) * P, :])
        for (dst, srcap) in ((bv_bc, io["bv_rowh"]), (bo_bc, io["bo_rowh"])):
            bc = bass.AP(tensor=srcap.tensor, offset=srcap.offset,
                         ap=[[0, P]] + list(srcap.ap[1:]))
            nc.sync.dma_start(out=dst, in_=bc)
        for k in range(EC):
            nc.sync.dma_start(out=wo_sb[k],
                              in_=io["wo_t"][k * P:(k + 1) * P, :])

        # ---------------- emission helpers ----------------
        def kq_chunk(m):
            """K^T chunks (both 1024-token group pairs) + Q^T chunk for
            feature rows m*128:(m+1)*128 (heads 2m, 2m+1)."""
            msl = slice(m * P, (m + 1) * P)
            for gp in range(n_g // 2):
                ps = psp.tile([P, 1024], F32, name=f"ps_k{gp}_{m}",
                              tag="st", bufs=3)
                for gi in range(2):
                    for k in range(EC):
                        nc.tensor.matmul(ps[:, gi * 512:(gi + 1) * 512],
                                         lhsT=wk_sb[k][:, msl],
                                         rhs=xts[gp, k][:, gi * 512:(gi + 1) * 512],
                                         start=(k == 0), stop=(k == EC - 1))
                nc.scalar.activation(KT[m][:, gp * 1024:(gp + 1) * 1024],
                                     ps, AF.Gelu, bias=bk_sb[m])
            ps = psp.tile([P, 1024], F32, name=f"ps_q_{m}", tag="st", bufs=3)
            for gi in range(2):
                for k in range(EC):
                    nc.tensor.matmul(ps[:, gi * 512:(gi + 1) * 512],
                                     lhsT=wq_sb[k][:, msl],
                                     rhs=xts[0, k][:, gi * 512:(gi + 1) * 512],
                                     start=(k == 0), stop=(k == EC - 1))
            nc.scalar.activation(QT[m], ps, AF.Gelu, bias=bq_sb[m])

        def v_tiles():
            """V natural [token, feature] for all 16 token tiles; bias row
            seeded into PSUM by a K=1 ones matmul (keeps DVE free)."""
            for t in range(n_kt):
                gp, s8 = divmod(t, 8)
                ps = psp.tile([P, E], F32, name=f"ps_v{t}", tag="st", bufs=3)
                for k in range(EC):
                    nc.tensor.matmul(ps,
                                     lhsT=xts[gp, k][:, s8 * P:(s8 + 1) * P],
                                     rhs=wv_sb[k],
                                     start=(k == 0), stop=(k == EC - 1))
                nc.vector.tensor_add(ps, ps, bv_bc)
                vst = xtp.tile([P, E], mm_dt, name=f"vst{t}", tag="vst",
                               bufs=3)
                nc.scalar.activation(vst, ps, AF.Gelu)
                nc.sync.dma_start(
                    out=V3[t][:, :, 0:64],
                    in_=vst.rearrange("p (h d) -> p h d", h=H))

        def st_exp_unit(hp, q, jp):
            """S^T matmuls + exp for one (head pair, q-half, key-tile pair);
            returns the e tiles for the deferred PV emission."""
            qsl = slice(q * 512, (q + 1) * 512)
            st0 = psp.tile([P, 1024], F32, name=f"st0_{hp}{q}{jp}",
                           tag="st", bufs=3)
            st1 = psp.tile([P, 1024], F32, name=f"st1_{hp}{q}{jp}",
                           tag="st", bufs=3)
            # interleave even/odd so adjacent matmuls use disjoint PE
            # row groups (0:64 vs 64:128) and run concurrently.
            for u in range(2):
                kt = jp * 2 + u
                ksl = slice(kt * P, (kt + 1) * P)
                usl = slice(u * 512, (u + 1) * 512)
                nc.tensor.matmul(st0[:, usl], lhsT=KT[hp][0:64, ksl],
                                 rhs=QT[hp][0:64, qsl],
                                 start=True, stop=True)
                nc.tensor.matmul(st1[:, usl], lhsT=KT[hp][64:128, ksl],
                                 rhs=QT[hp][64:128, qsl],
                                 start=True, stop=True)
            e0 = ep.tile([P, 1024], mm_dt, name=f"e0_{hp}{q}{jp}", tag="e0")
            e1 = ep.tile([P, 1024], mm_dt, name=f"e1_{hp}{q}{jp}", tag="e1")
            # even head: exact exp on ScalarE; odd head: Schraudolph
            # bit-trick exp on VectorE (engine load balance).
            nc.scalar.activation(e0, st0, AF.Exp, scale=SCALE)
            nc.vector.tensor_scalar(
                out=e1.bitcast(I16), in0=st1,
                scalar1=float(SCH_A), scalar2=float(SCH_B),
                op0=ALU.mult, op1=ALU.add)
            return e0, e1

        def pv_unit(hp, q, jp, e0, e1):
            pv0, pv1 = pv[hp, q, 0], pv[hp, q, 1]
            he, ho = 2 * hp, 2 * hp + 1
            for u in range(2):
                kt = jp * 2 + u
                usl = slice(u * 512, (u + 1) * 512)
                nc.tensor.matmul(pv0, lhsT=V3[kt][:, he, 0:65],
                                 rhs=e0[:, usl],
                                 start=(kt == 0), stop=(kt == n_kt - 1))
            for u in range(2):
                kt = jp * 2 + u
                usl = slice(u * 512, (u + 1) * 512)
                nc.tensor.matmul(pv1, lhsT=V3[kt][:, ho, 0:65],
                                 rhs=e1[:, usl],
                                 start=(kt == 0), stop=(kt == n_kt - 1))

        def drain(hp, q, tail=False):
            """Move O^T_unnorm into ORT and sumexp rows into G.  Even head
            rows are partition-aligned; the odd head hops across partitions
            via SBUF + DMA.  The two big copies ride ScalarE (it has more
            per-unit slack than VectorE).  tail=True short-circuits the G
            gather: the reciprocal runs right here on the staging tiles and
            lands straight in SE, shortening the end-of-kernel norm chain."""
            pv0, pv1 = pv[hp, q, 0], pv[hp, q, 1]
            he, ho = 2 * hp, 2 * hp + 1
            qsl = slice(q * 512, (q + 1) * 512)
            cp = nc.scalar.copy if tail else \
                (lambda o, i: nc.vector.tensor_copy(o, i))
            cp(ORT[hp][0:64, qsl], pv0[0:64, :])
            tmp_v = tmpp.tile([64, 512], mm_dt, name=f"tv_{hp}_{q}", tag="tv")
            ts0 = tmpp.tile([65, 512], F32, name=f"ts0_{hp}_{q}", tag="ts0")
            ts1 = tmpp.tile([65, 512], F32, name=f"ts1_{hp}_{q}", tag="ts1")
            cp(tmp_v, pv1[0:64, :])
            nc.vector.tensor_copy(ts0[64:65, :], pv0[64:65, :])
            nc.vector.tensor_copy(ts1[64:65, :], pv1[64:65, :])
            nc.sync.dma_start(out=ORT[hp][64:128, qsl], in_=tmp_v)
            ge, re = g_loc(he)
            go, ro = g_loc(ho)
            nc.sync.dma_start(out=ge[re:re + 1, qsl], in_=ts0[64:65, :])
            nc.sync.dma_start(out=go[ro:ro + 1, qsl], in_=ts1[64:65, :])
            if tail:
                # reciprocal immediately (the custom DVE op only works at
                # base partition 0, hence via G) and straight into SE so the
                # end-of-kernel norm chain is short.
                gq = G[hp]
                nc.vector.reciprocal_approx_fast(out=gq[0:2, qsl],
                                                 in_=gq[0:2, qsl])
                nc.sync.dma_start(out=SE[hp][64:65, qsl],
                                  in_=gq[0:1, qsl])
                nc.sync.dma_start(out=SE[hp][32:33, qsl],
                                  in_=gq[1:2, qsl])

        def norm(hps, q, tail=False):
            """Normalize ORT[:, q-half] for the given head pairs (hps must
            be [0,1], [2,3] or [0,1,2,3] so the reciprocal APs start at
            partition 0): reciprocal over the G rows, partition broadcast
            via K=1 selector matmuls, DVE multiply.  tail=True means the
            drains already reciprocated into SE directly."""
            qsl = slice(q * 512, (q + 1) * 512)
            if not tail:
                for hp in hps:
                    nc.vector.reciprocal_approx_fast(out=G[hp][0:2, qsl],
                                                     in_=G[hp][0:2, qsl])
                for hp in hps:
                    nc.sync.dma_start(out=SE[hp][64:65, qsl],
                                      in_=G[hp][0:1, qsl])
                    nc.sync.dma_start(out=SE[hp][32:33, qsl],
                                      in_=G[hp][1:2, qsl])
            for hp in hps:
                R = psp.tile([P, 512], F32, name=f"R_{hp}_{q}", tag="st",
                             bufs=3)
                nc.tensor.matmul(R[0:64, :], lhsT=sel2[64:65, 0:64],
                                 rhs=SE[hp][64:65, qsl],
                                 start=True, stop=True)
                nc.tensor.matmul(R[64:128, :], lhsT=sel2[32:33, 0:64],
                                 rhs=SE[hp][32:33, qsl],
                                 start=True, stop=True)
                nc.vector.tensor_mul(ORT[hp][:, qsl], ORT[hp][:, qsl], R)

        def out_proj(ts):
            for t in ts:
                tsl = slice(t * P, (t + 1) * P)
                ps = psp.tile([P, E], F32, name=f"ps_o{t}", tag="st", bufs=3)
                for m in range(EC):
                    nc.tensor.matmul(ps, lhsT=ORT[m][:, tsl], rhs=wo_sb[m],
                                     start=(m == 0), stop=(m == EC - 1))
                nc.vector.tensor_add(ps, ps, bo_bc)
                ot = outp.tile([P, E], F32, name=f"ot_{t}", tag="ot")
                nc.scalar.activation(ot, ps, AF.Gelu)
                # alternate queues so the eight 256KB stores don't serialize
                eng = nc.sync if t % 2 == 0 else nc.scalar
                eng.dma_start(out=out[tsl, :], in_=ot)

        # ---------------- program ----------------
        pv = {}
        for q in range(n_qg):
            for hp in range(H // 2):
                pv[hp, q, 0] = psp.tile([65, 512], F32, name=f"pv0_{hp}_{q}",
                                        tag="pv", bufs=2)
                pv[hp, q, 1] = psp.tile([65, 512], F32, name=f"pv1_{hp}_{q}",
                                        tag="pv", bufs=2)

        # HAM warmup: ~3.5us of junk matmuls while the input DMAs land so
        # the PE clock is at 2.4 GHz when the first projection runs.
        warm = persist.tile([64, 256], mm_dt, name="warm")
        nc.vector.memset(warm, 0.5)
        wps = psp.tile([64, 512], F32, name="warm_ps", tag="st", bufs=3)
        for _ in range(36):
            nc.tensor.matmul(wps[:, 0:128], lhsT=warm[:, 0:64],
                             rhs=warm[:, 0:128], start=True, stop=True)

        kq_chunk(0)
        v_tiles()
        kq_chunk(1)
        kq_chunk(2)
        kq_chunk(3)

        # Attention stream, software-pipelined with a one-unit lag: each
        # unit's PV matmuls are emitted after the NEXT unit's S^T matmuls so
        # the PE never sits waiting for the exp of the tile it just built.
        pend = None

        def run_block(hp, q, tail=False):
            nonlocal pend
            for jp in range(n_kt // 2):
                e0, e1 = st_exp_unit(hp, q, jp)
                if pend is not None:
                    pend()
                last = jp == n_kt // 2 - 1
                pend = (lambda hp=hp, q=q, jp=jp, e0=e0, e1=e1, last=last,
                        tail=tail:
                        (pv_unit(hp, q, jp, e0, e1),
                         drain(hp, q, tail=tail) if last else None))

        def flush():
            nonlocal pend
            if pend is not None:
                pend()
                pend = None

        run_block(0, 0)
        run_block(1, 0)
        run_block(2, 0)
        run_block(3, 0)
        run_block(0, 1)
        run_block(1, 1)
        norm([0, 1, 2, 3], 0)
        run_block(2, 1, tail=True)
        norm([0, 1], 1)
        run_block(3, 1, tail=True)
        flush()
        # out tiles 0..3 only need the long-finished q0 normalization —
        # emit them ahead of the tail norm so the PE works through them
        # while the (2,3)/q1 reciprocal chain resolves.
        out_proj(range(0, 4))
        norm([2, 3], 1, tail=True)
        out_proj(range(4, 8))


def make_nc(T_KV, T_Q, num_devices=N_CORES, mm_dt=BF16, debug=False):
    nc = bacc.Bacc("TRN2", target_bir_lowering=False, debug=debug,
                   num_devices=num_devices)
    io = {
        "xkv_t": nc.dram_tensor("xkv_t", [E, T_KV], mm_dt,
                                kind="ExternalInput").ap(),
        "wq_t": nc.dram_tensor("wq_t", [E, E], mm_dt,
                               kind="ExternalInput").ap(),
        "wk_t": nc.dram_tensor("wk_t", [E, E], mm_dt,
                               kind="ExternalInput").ap(),
        "wv_t": nc.dram_tensor("wv_t", [E, E], mm_dt,
                               kind="ExternalInput").ap(),
        "wo_t": nc.dram_tensor("wo_t", [E, E], mm_dt,
                               kind="ExternalInput").ap(),
        "bq_col": nc.dram_tensor("bq_col", [E, 1], F32,
                                 kind="ExternalInput").ap(),
        "bk_col": nc.dram_tensor("bk_col", [E, 1], F32,
                                 kind="ExternalInput").ap(),
        "bv_rowh": nc.dram_tensor("bv_rowh", [1, E], mm_dt,
                                  kind="ExternalInput").ap(),
        "bo_rowh": nc.dram_tensor("bo_rowh", [1, E], mm_dt,
                                  kind="ExternalInput").ap(),
        "out": nc.dram_tensor("out", [T_Q, E], F32, kind="ExternalOutput").ap(),
    }
    with tile.TileContext(nc) as tc:
        build(tc, io, T_KV=T_KV, T_Q=T_Q, mm_dt=mm_dt)
    nc.compile()
    return nc


def make_in_maps(x, Wq, bq, Wk, bk, Wv, bv, Wo, bo, mm_np=None):
    if mm_np is None:
        import ml_dtypes
        mm_np = ml_dtypes.bfloat16
    castm = lambda a: np.ascontiguousarray(np.asarray(a).astype(mm_np))
    castf = lambda a: np.ascontiguousarray(np.asarray(a, dtype=np.float32))
    base = {
        "wq_t": castm(np.asarray(Wq).T),
        "wk_t": castm(np.asarray(Wk).T),
        "wv_t": castm(np.asarray(Wv).T),
        "wo_t": castm(np.asarray(Wo).T),
        "bq_col": castf(np.asarray(bq)[:, None]),
        "bk_col": castf(np.asarray(bk)[:, None]),
        "bv_rowh": castm(np.asarray(bv)[None, :]),
        "bo_rowh": castm(np.asarray(bo)[None, :]),
    }
    x = np.asarray(x)
    half_len = S // 2
    in_maps = []
    for c in range(N_CORES):
        b, half = divmod(c, 2)
        xb = x[b]
        mine = xb[half * half_len:(half + 1) * half_len]
        oth = xb[(1 - half) * half_len:(2 - half) * half_len]
        m = dict(base)
        m["xkv_t"] = castm(np.concatenate([mine, oth], axis=0).T)
        in_maps.append(m)
    return in_maps


_NC_CACHE = {}


def _get_full_nc():
    if "full" not in _NC_CACHE:
        _NC_CACHE["full"] = make_nc(T_KV=S, T_Q=S // 2)
    return _NC_CACHE["full"]


def run_on_hw(in_maps, trace=False, **kw):
    from concourse.bass_utils import run_bass_kernel_spmd
    nc = _get_full_nc()
    return run_bass_kernel_spmd(nc, in_maps, core_ids=list(range(N_CORES)),
                                trace=trace, **kw)


def kernel(x, Wq, bq, Wk, bk, Wv, bv, Wo, bo):
    in_maps = make_in_maps(x, Wq, bq, Wk, bk, Wv, bv, Wo, bo)
    res = run_on_hw(in_maps)
    half_len = S // 2
    out = np.empty((B, S, E), np.float32)
    for c in range(N_CORES):
        b, half = divmod(c, 2)
        out[b, half * half_len:(half + 1) * half_len, :] = \
            res.results[c]["out"]
    return out


# revision 40
# speedup vs baseline: 1.0232x; 1.0232x over previous
"""Trainium2 Bass kernel for nn_AttentionBlock (B=4, S=2048, E=512, H=8).

Sharding (no cross-core communication):
  core c owns batch b = c//2 and output rows for tokens half = c%2
  (1024 tokens).  Each core computes Q for its own 1024 tokens and K/V for
  the full batch (2048 tokens); K/V projections are duplicated between the
  two cores of a batch (cheap) so attention and the output projection are
  fully local.

Host-side layout prep (inside kernel(), numpy only):
  - x[b] is permuted so the core's own 1024 query tokens are rows 0:1024,
    then transposed to feature-major xkv_t [512, 2048].  Softmax/PV are
    permutation-invariant in the key axis, so K/V token order is irrelevant.
  - weights are passed as W.T [E, E] so lhsT/rhs tiles are direct DMA loads.
  - x and weights are cast to bf16; biases, PSUM accumulation, softmax
    statistics and the normalization stay fp32 (V/out bias rows are bf16
    broadcast tiles added on the DVE — error ~2e-4 absolute, negligible).

Device dataflow (per core):
  The exp of the score matrix (16.8M elements/core) saturates ScalarE if it
  owns all of it, so the exp is split: even head's tile on ScalarE (exact
  LUT exp), odd head's on VectorE via a Schraudolph bit-trick (bf16 bits =
  int16(A*s + B), one fused tensor_scalar from PSUM; numerator and
  denominator use the same approximate weights so softmax normalization
  cancels nearly all the error — measured end-to-end rel-err 2.9e-3).

  A) ~3.5us of junk warmup matmuls (HAM clock un-throttle) under the input
     DMAs, then projections: K/Q feature chunk m=0, V (16 token tiles, DVE
     bias add), K/Q m=1..3.  All gelus precede all exps so the ACT table
     set loads exactly 3 times (gelu, exp, gelu).
  B) attention, q in 512-column halves, head pairs hp inside: per (hp,q,jp)
     unit two K=64 S^T matmuls per head interleaved even/odd so adjacent
     matmuls land in disjoint PE row groups (auto-pack, 2x), exp split
     ACT/DVE, PV accumulates O^T_unnorm + sumexp via [V | 1] lhsT (M=65).
     The stream is software-pipelined with a one-unit lag (each unit's PV
     matmuls are emitted after the next unit's S^T) so the PE never waits
     on the exp of the tile it just produced.
  C) normalization incrementally (q0 batched under q1's attention; q1 in
     hp chunks as drains land, with the tail reciprocals emitted inside the
     drains): reciprocal_approx_fast on sumexp rows (base partition 0 only
     — the custom DVE op breaks at other bases), K=1 selector matmuls
     broadcast across 64 partitions (separate accumulation groups per lhsT
     base partition — mixing crashes the HW), DVE multiply.
  D) out = gelu(ORT chunks @ Wo.T + bo) from PSUM (out tiles 0-3 emitted
     ahead of the tail norm chain); stores alternate sync/scalar queues.
"""

import numpy as np

import concourse.bass as bass
import concourse.tile as tile
from concourse import bacc, mybir

F32 = mybir.dt.float32
BF16 = mybir.dt.bfloat16
I16 = mybir.dt.int16
AF = mybir.ActivationFunctionType
ALU = mybir.AluOpType

E = 512          # embed dim
H = 8            # heads
D = 64           # head dim
P = 128          # partitions
EC = E // P      # 128-row chunks of the embed dim
B = 4
S = 2048
N_CORES = 8
SCALE = 0.125    # 1/sqrt(D)

# Schraudolph exp-as-bit-trick constants: bf16 bits of exp(s/8) ~=
# int16(SCH_A*s + SCH_B).  C=0.045 centers the mantissa sawtooth error.
SCH_A = 128.0 * SCALE * np.log2(np.e)       # 23.08312...
SCH_B = 128.0 * (127.0 - 0.045)             # 16250.24


def build(tc, io, T_KV, T_Q, mm_dt=BF16):
    """Emit the per-core program.  T_KV: key/value tokens; T_Q: query tokens
    (the first T_Q columns of xkv_t)."""
    nc = tc.nc
    n_g = T_KV // 512        # 512-token groups for projections
    n_qg = T_Q // 512        # q groups (S^T rhs is N=512)
    n_kt = T_KV // P         # 128-token key tiles
    assert T_KV % 1024 == 0 and T_Q % 1024 == 0

    xkv = io["xkv_t"]
    out = io["out"]

    with tc.tile_pool(name="persist", bufs=1) as persist, \
         tc.tile_pool(name="ps", space="PSUM", bufs=1) as psp, \
         tc.tile_pool(name="projw", bufs=1) as projw, \
         tc.tile_pool(name="xtp", bufs=2) as xtp, \
         tc.tile_pool(name="ep", bufs=4) as ep, \
         tc.tile_pool(name="tmpp", bufs=2) as tmpp, \
         tc.tile_pool(name="outp", bufs=3) as outp:
        # ---------------- persistent SBUF state ----------------
        wo_sb = [persist.tile([P, E], mm_dt, name=f"wo_sb{k}", tag=f"wo{k}")
                 for k in range(EC)]
        QT = [persist.tile([P, T_Q], mm_dt, name=f"qt_sb{m}", tag=f"qt{m}")
              for m in range(EC)]
        KT = [persist.tile([P, T_KV], mm_dt, name=f"kt_sb{m}", tag=f"kt{m}")
              for m in range(EC)]
        V3 = [persist.tile([P, H, 66], mm_dt, name=f"v3_sb{t}", tag=f"v3{t}")
              for t in range(n_kt)]
        ORT = [persist.tile([P, T_Q], mm_dt, name=f"ort_sb{m}", tag=f"ort{m}")
               for m in range(EC)]
        # sumexp rows live at partition 64 (even head) and 32 (odd head):
        # SBUF accesses may only start at partitions {0, 32, 64, 96} and
        # matmul operand base partitions only at {0, 32, 64}.
        SE = [persist.tile([65, T_Q], F32, name=f"se_sb{hp}", tag=f"se{hp}")
              for hp in range(H // 2)]
        # sumexp rows gathered per head pair so every DVE reciprocal runs at
        # base partition 0 (compute-engine SBUF accesses must start at
        # partition {0,32,64,96}, and the custom reciprocal op is only
        # proven at base 0).
        G = [persist.tile([2, T_Q], F32, name=f"g_se{i}") for i in range(4)]
        g_loc = lambda h: (G[h // 2], h % 2)
        sel2 = persist.tile([65, P], F32, name="sel2")
        bv_bc = persist.tile([P, E], mm_dt, name="bv_bc")
        bo_bc = persist.tile([P, E], mm_dt, name="bo_bc")

        # sel2 rows 64 / 32 are 64-wide ones rows used as K=1 lhsT to
        # broadcast the even / odd head's reciprocal sumexp across 64
        # output partitions.
        nc.vector.memset(sel2[64:65, 0:64], 1.0)
        nc.vector.memset(sel2[32:33, 0:64], 1.0)
        # bf16 strided writes (ACT out / gpsimd memset with a gappy AP)
        # wedge the device — memset the whole tile (contiguous) instead;
        # the V gelu lands via a contiguous staging tile + strided DMA.
        for t in range(n_kt):
            nc.gpsimd.memset(V3[t], 1.0)

        # ---------------- weight / x DMA issue ----------------
        wq_sb = [projw.tile([P, E], mm_dt, name=f"wq_sb{k}", tag=f"wq{k}")
                 for k in range(EC)]
        wk_sb = [projw.tile([P, E], mm_dt, name=f"wk_sb{k}", tag=f"wk{k}")
                 for k in range(EC)]
        wv_sb = [projw.tile([P, E], mm_dt, name=f"wv_sb{k}", tag=f"wv{k}")
                 for k in range(EC)]
        bq_sb = [projw.tile([P, 1], F32, name=f"bq_sb{m}", tag=f"bq{m}")
                 for m in range(EC)]
        bk_sb = [projw.tile([P, 1], F32, name=f"bk_sb{m}", tag=f"bk{m}")
                 for m in range(EC)]
        # x tiles merged to [P, 1024] (one DMA per (group pair, k-chunk));
        # issued on the scalar / vector queues so the startup loads don't
        # serialize behind the weight loads on sync.
        xts = {}
        for gp in range(n_g // 2):
            for k in range(EC):
                xts[gp, k] = xtp.tile([P, 1024], mm_dt, name=f"xt_g{gp}k{k}",
                                      tag=f"xt_{k}")
        nc.scalar.dma_start(out=xts[0, 0][:, 0:512],
                            in_=xkv[0:P, 0:512])
        nc.scalar.dma_start(out=xts[0, 0][:, 512:1024],
                            in_=xkv[0:P, 512:1024])
        for k in range(1, EC):
            nc.scalar.dma_start(out=xts[0, k],
                                in_=xkv[k * P:(k + 1) * P, 0:1024])
        for k in range(EC):
            nc.scalar.dma_start(out=xts[1, k],
                                in_=xkv[k * P:(k + 1) * P, 1024:2048])
        for k in range(EC):
            nc.sync.dma_start(out=wk_sb[k],
                              in_=io["wk_t"][k * P:(k + 1) * P, :])
        for k in range(EC):
            nc.sync.dma_start(out=wq_sb[k],
                              in_=io["wq_t"][k * P:(k + 1) * P, :])
        for k in range(EC):
            nc.sync.dma_start(out=bq_sb[k],
                              in_=io["bq_col"][k * P:(k + 1) * P, :])
            nc.sync.dma_start(out=bk_sb[k],
                              in_=io["bk_col"][k * P:(k + 1) * P, :])
        for k in range(EC):
            nc.sync.dma_start(out=wv_sb[k],
                              in_=io["wv_t"][k * P:(k + 1) * P, :])
        for (dst, srcap) in ((bv_bc, io["bv_rowh"]), (bo_bc, io["bo_rowh"])):
            bc = bass.AP(tensor=srcap.tensor, offset=srcap.offset,
                         ap=[[0, P]] + list(srcap.ap[1:]))
            nc.sync.dma_start(out=dst, in_=bc)
        for k in range(EC):
            nc.sync.dma_start(out=wo_sb[k],
                              in_=io["wo_t"][k * P:(k + 1) * P, :])

        # ---------------- emission helpers ----------------
        def kq_chunk(m):
            """K^T chunks (both 1024-token group pairs) + Q^T chunk for
            feature rows m*128:(m+1)*128 (heads 2m, 2m+1)."""
            msl = slice(m * P, (m + 1) * P)
            for gp in range(n_g // 2):
                ps = psp.tile([P, 1024], F32, name=f"ps_k{gp}_{m}",
                              tag="st", bufs=3)
                for gi in range(2):
                    for k in range(EC):
                        nc.tensor.matmul(ps[:, gi * 512:(gi + 1) * 512],
                                         lhsT=wk_sb[k][:, msl],
                                         rhs=xts[gp, k][:, gi * 512:(gi + 1) * 512],
                                         start=(k == 0), stop=(k == EC - 1))
                nc.scalar.activation(KT[m][:, gp * 1024:(gp + 1) * 1024],
                                     ps, AF.Gelu, bias=bk_sb[m])
            ps = psp.tile([P, 1024], F32, name=f"ps_q_{m}", tag="st", bufs=3)
            for gi in range(2):
                for k in range(EC):
                    nc.tensor.matmul(ps[:, gi * 512:(gi + 1) * 512],
                                     lhsT=wq_sb[k][:, msl],
                                     rhs=xts[0, k][:, gi * 512:(gi + 1) * 512],
                                     start=(k == 0), stop=(k == EC - 1))
            nc.scalar.activation(QT[m], ps, AF.Gelu, bias=bq_sb[m])

        def v_tiles():
            """V natural [token, feature] for all 16 token tiles; bias row
            seeded into PSUM by a K=1 ones matmul (keeps DVE free)."""
            for t in range(n_kt):
                gp, s8 = divmod(t, 8)
                ps = psp.tile([P, E], F32, name=f"ps_v{t}", tag="st", bufs=3)
                for k in range(EC):
                    nc.tensor.matmul(ps,
                                     lhsT=xts[gp, k][:, s8 * P:(s8 + 1) * P],
                                     rhs=wv_sb[k],
                                     start=(k == 0), stop=(k == EC - 1))
                nc.vector.tensor_add(ps, ps, bv_bc)
                vst = xtp.tile([P, E], mm_dt, name=f"vst{t}", tag="vst",
                               bufs=3)
                nc.scalar.activation(vst, ps, AF.Gelu)
                nc.sync.dma_start(
                    out=V3[t][:, :, 0:64],
                    in_=vst.rearrange("p (h d) -> p h d", h=H))

        def st_exp_unit(hp, q, jp):
            """S^T matmuls + exp for one (head pair, q-half, key-tile pair);
            returns the e tiles for the deferred PV emission."""
            qsl = slice(q * 512, (q + 1) * 512)
            st0 = psp.tile([P, 1024], F32, name=f"st0_{hp}{q}{jp}",
                           tag="st", bufs=3)
            st1 = psp.tile([P, 1024], F32, name=f"st1_{hp}{q}{jp}",
                           tag="st", bufs=3)
            # interleave even/odd so adjacent matmuls use disjoint PE
            # row groups (0:64 vs 64:128) and run concurrently.
            for u in range(2):
                kt = jp * 2 + u
                ksl = slice(kt * P, (kt + 1) * P)
                usl = slice(u * 512, (u + 1) * 512)
                nc.tensor.matmul(st0[:, usl], lhsT=KT[hp][0:64, ksl],
                                 rhs=QT[hp][0:64, qsl],
                                 start=True, stop=True)
                nc.tensor.matmul(st1[:, usl], lhsT=KT[hp][64:128, ksl],
                                 rhs=QT[hp][64:128, qsl],
                                 start=True, stop=True)
            e0 = ep.tile([P, 1024], mm_dt, name=f"e0_{hp}{q}{jp}", tag="e0")
            e1 = ep.tile([P, 1024], mm_dt, name=f"e1_{hp}{q}{jp}", tag="e1")
            # even head: exact exp on ScalarE; odd head: Schraudolph
            # bit-trick exp on VectorE (engine load balance).
            nc.scalar.activation(e0, st0, AF.Exp, scale=SCALE)
            nc.vector.tensor_scalar(
                out=e1.bitcast(I16), in0=st1,
                scalar1=float(SCH_A), scalar2=float(SCH_B),
                op0=ALU.mult, op1=ALU.add)
            return e0, e1

        def pv_unit(hp, q, jp, e0, e1):
            pv0, pv1 = pv[hp, q, 0], pv[hp, q, 1]
            he, ho = 2 * hp, 2 * hp + 1
            for u in range(2):
                kt = jp * 2 + u
                usl = slice(u * 512, (u + 1) * 512)
                nc.tensor.matmul(pv0, lhsT=V3[kt][:, he, 0:65],
                                 rhs=e0[:, usl],
                                 start=(kt == 0), stop=(kt == n_kt - 1))
            for u in range(2):
                kt = jp * 2 + u
                usl = slice(u * 512, (u + 1) * 512)
                nc.tensor.matmul(pv1, lhsT=V3[kt][:, ho, 0:65],
                                 rhs=e1[:, usl],
                                 start=(kt == 0), stop=(kt == n_kt - 1))

        def drain(hp, q, tail=False):
            """Move O^T_unnorm into ORT and sumexp rows into G.  Even head
            rows are partition-aligned; the odd head hops across partitions
            via SBUF + DMA.  The two big copies ride ScalarE (it has more
            per-unit slack than VectorE).  tail=True short-circuits the G
            gather: the reciprocal runs right here on the staging tiles and
            lands straight in SE, shortening the end-of-kernel norm chain."""
            pv0, pv1 = pv[hp, q, 0], pv[hp, q, 1]
            he, ho = 2 * hp, 2 * hp + 1
            qsl = slice(q * 512, (q + 1) * 512)
            cp = nc.scalar.copy if tail else \
                (lambda o, i: nc.vector.tensor_copy(o, i))
            cp(ORT[hp][0:64, qsl], pv0[0:64, :])
            tmp_v = tmpp.tile([64, 512], mm_dt, name=f"tv_{hp}_{q}", tag="tv")
            ts0 = tmpp.tile([65, 512], F32, name=f"ts0_{hp}_{q}", tag="ts0")
            ts1 = tmpp.tile([65, 512], F32, name=f"ts1_{hp}_{q}", tag="ts1")
            cp(tmp_v, pv1[0:64, :])
            nc.vector.tensor_copy(ts0[64:65, :], pv0[64:65, :])
            nc.vector.tensor_copy(ts1[64:65, :], pv1[64:65, :])
            nc.sync.dma_start(out=ORT[hp][64:128, qsl], in_=tmp_v)
            ge, re = g_loc(he)
            go, ro = g_loc(ho)
            nc.sync.dma_start(out=ge[re:re + 1, qsl], in_=ts0[64:65, :])
            nc.sync.dma_start(out=go[ro:ro + 1, qsl], in_=ts1[64:65, :])
            if tail:
                # reciprocal immediately (the custom DVE op only works at
                # base partition 0, hence via G) and straight into SE so the
                # end-of-kernel norm chain is short.
                gq = G[hp]
                nc.vector.reciprocal_approx_fast(out=gq[0:2, qsl],
                                                 in_=gq[0:2, qsl])
                nc.sync.dma_start(out=SE[hp][64:65, qsl],
                                  in_=gq[0:1, qsl])
                nc.sync.dma_start(out=SE[hp][32:33, qsl],
                                  in_=gq[1:2, qsl])

        def norm(hps, q, tail=False):
            """Normalize ORT[:, q-half] for the given head pairs (hps must
            be [0,1], [2,3] or [0,1,2,3] so the reciprocal APs start at
            partition 0): reciprocal over the G rows, partition broadcast
            via K=1 selector matmuls, DVE multiply.  tail=True means the
            drains already reciprocated into SE directly."""
            qsl = slice(q * 512, (q + 1) * 512)
            if not tail:
                for hp in hps:
                    nc.vector.reciprocal_approx_fast(out=G[hp][0:2, qsl],
                                                     in_=G[hp][0:2, qsl])
                for hp in hps:
                    nc.sync.dma_start(out=SE[hp][64:65, qsl],
                                      in_=G[hp][0:1, qsl])
                    nc.sync.dma_start(out=SE[hp][32:33, qsl],
                                      in_=G[hp][1:2, qsl])
            for hp in hps:
                R = psp.tile([P, 512], F32, name=f"R_{hp}_{q}", tag="st",
                             bufs=3)
                nc.tensor.matmul(R[0:64, :], lhsT=sel2[64:65, 0:64],
                                 rhs=SE[hp][64:65, qsl],
                                 start=True, stop=True)
                nc.tensor.matmul(R[64:128, :], lhsT=sel2[32:33, 0:64],
                                 rhs=SE[hp][32:33, qsl],
                                 start=True, stop=True)
                nc.vector.tensor_mul(ORT[hp][:, qsl], ORT[hp][:, qsl], R)

        def out_proj(ts):
            for t in ts:
                tsl = slice(t * P, (t + 1) * P)
                ps = psp.tile([P, E], F32, name=f"ps_o{t}", tag="st", bufs=3)
                for m in range(EC):
                    nc.tensor.matmul(ps, lhsT=ORT[m][:, tsl], rhs=wo_sb[m],
                                     start=(m == 0), stop=(m == EC - 1))
                nc.vector.tensor_add(ps, ps, bo_bc)
                ot = outp.tile([P, E], F32, name=f"ot_{t}", tag="ot")
                nc.scalar.activation(ot, ps, AF.Gelu)
                # alternate queues so the eight 256KB stores don't serialize
                eng = nc.sync if t % 2 == 0 else nc.scalar
                eng.dma_start(out=out[tsl, :], in_=ot)

        # ---------------- program ----------------
        pv = {}
        for q in range(n_qg):
            for hp in range(H // 2):
                pv[hp, q, 0] = psp.tile([65, 512], F32, name=f"pv0_{hp}_{q}",
                                        tag="pv", bufs=2)
                pv[hp, q, 1] = psp.tile([65, 512], F32, name=f"pv1_{hp}_{q}",
                                        tag="pv", bufs=2)

        # HAM warmup: ~3.5us of junk matmuls while the input DMAs land so
        # the PE clock is at 2.4 GHz when the first projection runs.
        warm = persist.tile([64, 256], mm_dt, name="warm")
        nc.vector.memset(warm, 0.5)
        wps = psp.tile([64, 512], F32, name="warm_ps", tag="st", bufs=3)
        for _ in range(40):
            nc.tensor.matmul(wps[:, 0:128], lhsT=warm[:, 0:64],
                             rhs=warm[:, 0:128], start=True, stop=True)

        kq_chunk(0)
        v_tiles()
        kq_chunk(1)
        kq_chunk(2)
        kq_chunk(3)

        # Attention stream, software-pipelined with a one-unit lag: each
        # unit's PV matmuls are emitted after the NEXT unit's S^T matmuls so
        # the PE never sits waiting for the exp of the tile it just built.
        pend = None

        def run_block(hp, q, tail=False):
            nonlocal pend
            for jp in range(n_kt // 2):
                e0, e1 = st_exp_unit(hp, q, jp)
                if pend is not None:
                    pend()
                last = jp == n_kt // 2 - 1
                pend = (lambda hp=hp, q=q, jp=jp, e0=e0, e1=e1, last=last,
                        tail=tail:
                        (pv_unit(hp, q, jp, e0, e1),
                         drain(hp, q, tail=tail) if last else None))

        def flush():
            nonlocal pend
            if pend is not None:
                pend()
                pend = None

        run_block(0, 0)
        run_block(1, 0)
        run_block(2, 0)
        run_block(3, 0)
        run_block(0, 1)
        run_block(1, 1)
        norm([0, 1, 2, 3], 0)
        run_block(2, 1, tail=True)
        norm([0, 1], 1)
        run_block(3, 1, tail=True)
        flush()
        # out tiles 0..3 only need the long-finished q0 normalization —
        # emit them ahead of the tail norm so the PE works through them
        # while the (2,3)/q1 reciprocal chain resolves.
        out_proj(range(0, 4))
        norm([2, 3], 1, tail=True)
        out_proj(range(4, 8))


def make_nc(T_KV, T_Q, num_devices=N_CORES, mm_dt=BF16, debug=False):
    nc = bacc.Bacc("TRN2", target_bir_lowering=False, debug=debug,
                   num_devices=num_devices)
    io = {
        "xkv_t": nc.dram_tensor("xkv_t", [E, T_KV], mm_dt,
                                kind="ExternalInput").ap(),
        "wq_t": nc.dram_tensor("wq_t", [E, E], mm_dt,
                               kind="ExternalInput").ap(),
        "wk_t": nc.dram_tensor("wk_t", [E, E], mm_dt,
                               kind="ExternalInput").ap(),
        "wv_t": nc.dram_tensor("wv_t", [E, E], mm_dt,
                               kind="ExternalInput").ap(),
        "wo_t": nc.dram_tensor("wo_t", [E, E], mm_dt,
                               kind="ExternalInput").ap(),
        "bq_col": nc.dram_tensor("bq_col", [E, 1], F32,
                                 kind="ExternalInput").ap(),
        "bk_col": nc.dram_tensor("bk_col", [E, 1], F32,
                                 kind="ExternalInput").ap(),
        "bv_rowh": nc.dram_tensor("bv_rowh", [1, E], mm_dt,
                                  kind="ExternalInput").ap(),
        "bo_rowh": nc.dram_tensor("bo_rowh", [1, E], mm_dt,
                                  kind="ExternalInput").ap(),
        "out": nc.dram_tensor("out", [T_Q, E], F32, kind="ExternalOutput").ap(),
    }
    with tile.TileContext(nc) as tc:
        build(tc, io, T_KV=T_KV, T_Q=T_Q, mm_dt=mm_dt)
    nc.compile()
    return nc


def make_in_maps(x, Wq, bq, Wk, bk, Wv, bv, Wo, bo, mm_np=None):
    if mm_np is None:
        import ml_dtypes
        mm_np = ml_dtypes.bfloat16
    castm = lambda a: np.ascontiguousarray(np.asarray(a).astype(mm_np))
    castf = lambda a: np.ascontiguousarray(np.asarray(a, dtype=np.float32))
    base = {
        "wq_t": castm(np.asarray(Wq).T),
        "wk_t": castm(np.asarray(Wk).T),
        "wv_t": castm(np.asarray(Wv).T),
        "wo_t": castm(np.asarray(Wo).T),
        "bq_col": castf(np.asarray(bq)[:, None]),
        "bk_col": castf(np.asarray(bk)[:, None]),
        "bv_rowh": castm(np.asarray(bv)[None, :]),
        "bo_rowh": castm(np.asarray(bo)[None, :]),
    }
    x = np.asarray(x)
    half_len = S // 2
    in_maps = []
    for c in range(N_CORES):
        b, half = divmod(c, 2)
        xb = x[b]
        mine = xb[half * half_len:(half + 1) * half_len]
        oth = xb[(1 - half) * half_len:(2 - half) * half_len]
        m = dict(base)
        m["xkv_t"] = castm(np.concatenate([mine, oth], axis=0).T)
        in_maps.append(m)
    return in_maps


_NC_CACHE = {}


def _get_full_nc():
    if "full" not in _NC_CACHE:
        _NC_CACHE["full"] = make_nc(T_KV=S, T_Q=S // 2)
    return _NC_CACHE["full"]


def run_on_hw(in_maps, trace=False, **kw):
    from concourse.bass_utils import run_bass_kernel_spmd
    nc = _get_full_nc()
    return run_bass_kernel_spmd(nc, in_maps, core_ids=list(range(N_CORES)),
                                trace=trace, **kw)


def kernel(x, Wq, bq, Wk, bk, Wv, bv, Wo, bo):
    in_maps = make_in_maps(x, Wq, bq, Wk, bk, Wv, bv, Wo, bo)
    res = run_on_hw(in_maps)
    half_len = S // 2
    out = np.empty((B, S, E), np.float32)
    for c in range(N_CORES):
        b, half = divmod(c, 2)
        out[b, half * half_len:(half + 1) * half_len, :] = \
            res.results[c]["out"]
    return out
